# revision 7
# baseline (speedup 1.0000x reference)
"""ConvCrossAttention Trainium2 kernel (Bass/Tile), SPMD over 8 NeuronCores.

Sharding: pure data-parallel over batch (B=16 -> 2 images per core). Each core
runs the full two-stream cross-attention block for its 2 images; no collectives.

v2: full-fp16 dataflow (fp32r matmuls stream at ~2 cycles/col and trip the PE
power throttle; bf16 streams 1 col/cycle).  All matmul operands are bf16, PSUM
accumulation stays fp32.  Engine placement per the measured v1 trace:

  - depthwise 3x3 q-conv on DVE: x is stored column-padded ([32,34] rows, zero
    pad cols) plus an odd-offset copy, so every tap is a single contiguous 1-D
    scalar_tensor_tensor with 4B-aligned bf16 operands -> DVE 2x packed mode,
    no per-row turnaround.
  - depthwise stride-2 kv-conv on GPSIMD (strided reads defeat DVE packing;
    GPSIMD is otherwise idle).
  - pointwise convs on PE as before (Q: weight-stationary into [cout,pos],
    K: weight-stationary, V: x-stationary into [pos,cout]).
  - attention per head pair: dotsT on PE, exp on ACT (psum->sbuf bf16),
    denominator via ones-matmuls writing even head to psum rows 0-63 and odd
    head to rows 64-127 directly (no zero-padded weights), reciprocal on DVE,
    oT on PE with the same direct partition-offset writes (no zeroed v copy),
    normalization fused into the PSUM->SBUF eviction (DVE mult -> bf16).
  - psum->sbuf bias evictions (qT, kT, out) on ACT, which sits closer to PSUM.
  - V conv bias folded into the output conv bias on the host; BN folded into
    depthwise taps + pointwise bias.
"""

import numpy as np
import ml_dtypes
from contextlib import ExitStack

import concourse.bass as bass
import concourse.bacc as bacc
import concourse.tile as tile
import concourse.mybir as mybir
from concourse.bass_utils import run_bass_kernel_spmd

F32 = mybir.dt.float32
BF16 = mybir.dt.float16
AOP = mybir.AluOpType
AF = mybir.ActivationFunctionType

N_CORES = 8
B = 16
IMGS = B // N_CORES          # images per core
DIM = 256                    # conv in channels
HEADS, DH, INNER = 8, 64, 512
HW = 1024                    # 32*32 q positions
HWP = 1088                   # 32*34 column-padded positions
HWK = 256                    # 16*16 kv positions
EPS = 1e-5
SCALE = DH ** -0.5

NPBF = np.float16


# ---------------------------------------------------------------------------
# device kernel
# ---------------------------------------------------------------------------

def _build_module():
    nc = bacc.Bacc("TRN2", target_bir_lowering=False, debug=False)

    def inp(name, shape, dt=F32):
        return nc.dram_tensor(name, shape, dt, kind="ExternalInput")

    x_d = [inp(f"xs{s}", [IMGS, DIM, HWP], BF16) for s in range(2)]
    dwq_d = [inp(f"dwq{s}", [2, 128, 9]) for s in range(2)]
    dwkv_d = [inp(f"dwkv{s}", [2, 128, 9]) for s in range(2)]
    wq_d = [inp(f"wq{s}", [DIM, INNER], BF16) for s in range(2)]        # [cin, cout]
    wkv_d = [inp(f"wkv{s}", [DIM, 2 * INNER], BF16) for s in range(2)]
    bq_d = [inp(f"bq{s}", [4, 128]) for s in range(2)]                  # cout chunk-major
    bk_d = [inp(f"bk{s}", [4, 128]) for s in range(2)]
    wo_d = [inp(f"wo{s}", [INNER, DIM], BF16) for s in range(2)]        # [hd, c]
    bo_d = [inp(f"bo{s}", [2, 128]) for s in range(2)]
    ones_d = inp("ones_in", [128, 64], BF16)
    out_d = nc.dram_tensor("out", [2, IMGS, DIM, HW], F32, kind="ExternalOutput")

    with tile.TileContext(nc) as tc, ExitStack() as ctx:
        const = ctx.enter_context(tc.tile_pool(name="const", bufs=1))
        xepool = ctx.enter_context(tc.tile_pool(name="xepool", bufs=2))
        xopool = ctx.enter_context(tc.tile_pool(name="xopool", bufs=2))
        yqpool = ctx.enter_context(tc.tile_pool(name="yqpool", bufs=2))
        ykpool = ctx.enter_context(tc.tile_pool(name="ykpool", bufs=2))
        qTpool = ctx.enter_context(tc.tile_pool(name="qTpool", bufs=8))
        kTpool = ctx.enter_context(tc.tile_pool(name="kTpool", bufs=8))
        vpool = ctx.enter_context(tc.tile_pool(name="vpool", bufs=4))
        pTpool = ctx.enter_context(tc.tile_pool(name="pTpool", bufs=16))
        Rpool = ctx.enter_context(tc.tile_pool(name="Rpool", bufs=3))
        oTpool = ctx.enter_context(tc.tile_pool(name="oTpool", bufs=4))
        outpool = ctx.enter_context(tc.tile_pool(name="outpool", bufs=2))
        psA = ctx.enter_context(tc.tile_pool(name="psA", bufs=2, space="PSUM"))
        psD = ctx.enter_context(tc.tile_pool(name="psD", bufs=1, space="PSUM"))
        psB = ctx.enter_context(tc.tile_pool(name="psB", bufs=2, space="PSUM"))

        # ---- constants -----------------------------------------------------
        wq_sb, wkv_sb, wo_sb, dwq_sb, dwkv_sb, bq_sb, bk_sb, bo_sb = (
            [], [], [], [], [], [], [], [])
        for s in range(2):
            wq_sb.append([const.tile([128, INNER], BF16, tag=f"wq{s}{k}", name=f"wq{s}{k}")
                          for k in range(2)])
            wkv_sb.append([const.tile([128, 2 * INNER], BF16, tag=f"wkv{s}{k}", name=f"wkv{s}{k}")
                           for k in range(2)])
            wo_sb.append([const.tile([128, DIM], BF16, tag=f"wo{s}{m}", name=f"wo{s}{m}")
                          for m in range(4)])
            for k in range(2):
                nc.sync.dma_start(out=wq_sb[s][k][:],
                                  in_=wq_d[s].ap()[k * 128:(k + 1) * 128, :])
                nc.sync.dma_start(out=wkv_sb[s][k][:],
                                  in_=wkv_d[s].ap()[k * 128:(k + 1) * 128, :])
            for m in range(4):
                nc.sync.dma_start(out=wo_sb[s][m][:],
                                  in_=wo_d[s].ap()[m * 128:(m + 1) * 128, :])
            dwq_sb.append(const.tile([128, 2, 9], F32, tag=f"dwq{s}", name=f"dwq{s}"))
            dwkv_sb.append(const.tile([128, 2, 9], F32, tag=f"dwkv{s}", name=f"dwkv{s}"))
            nc.sync.dma_start(out=dwq_sb[s][:],
                              in_=dwq_d[s].ap().rearrange("c p t -> p c t"))
            nc.sync.dma_start(out=dwkv_sb[s][:],
                              in_=dwkv_d[s].ap().rearrange("c p t -> p c t"))
            bq_sb.append(const.tile([128, 4], F32, tag=f"bq{s}", name=f"bq{s}"))
            bk_sb.append(const.tile([128, 4], F32, tag=f"bk{s}", name=f"bk{s}"))
            bo_sb.append(const.tile([128, 2], F32, tag=f"bo{s}", name=f"bo{s}"))
            nc.sync.dma_start(out=bq_sb[s][:], in_=bq_d[s].ap().rearrange("m p -> p m"))
            nc.sync.dma_start(out=bk_sb[s][:], in_=bk_d[s].ap().rearrange("m p -> p m"))
            nc.sync.dma_start(out=bo_sb[s][:], in_=bo_d[s].ap().rearrange("m p -> p m"))
        ones_sb = const.tile([128, 64], BF16, tag="ones", name="ones")
        nc.sync.dma_start(out=ones_sb[:], in_=ones_d.ap())

        for img in range(IMGS):
            qT, kT, v = {}, {}, {}
            # ---- projections for both streams ------------------------------
            for s in range(2):
                yq, ykv = [], []
                for c in range(2):
                    xe_t = xepool.tile([128, HWP], BF16, tag="xe", name="xe")
                    xo_t = xopool.tile([128, HWP + 2], BF16, tag="xo", name="xo")
                    nc.sync.dma_start(out=xe_t[:],
                                      in_=x_d[s].ap()[img, c * 128:(c + 1) * 128, :])
                    nc.sync.dma_start(out=xo_t[:, 1:HWP + 1],
                                      in_=x_d[s].ap()[img, c * 128:(c + 1) * 128, :])
                    # q depthwise: all taps flat-1D on the padded layout.
                    yq_t = yqpool.tile([128, HWP], BF16, tag="yq", name="yq")
                    nc.vector.tensor_scalar_mul(yq_t[:, 0:HWP], xe_t[:, 0:HWP],
                                                dwq_sb[s][:, c, 4:5])
                    for t, (ky, kx) in enumerate((ky, kx) for ky in range(3)
                                                 for kx in range(3)):
                        if t == 4:
                            continue
                        sh = 34 * (ky - 1) + (kx - 1)
                        a0 = 34 * (1 if ky == 0 else 0) + (2 if sh in (-35, -1) else 0)
                        b0 = 34 * (31 if ky == 2 else 32) - (2 if sh in (35, 1) else 0)
                        if sh % 2 == 0:
                            in0 = xe_t[:, a0 + sh:b0 + sh]
                        else:
                            in0 = xo_t[:, a0 + sh + 1:b0 + sh + 1]
                        nc.vector.scalar_tensor_tensor(
                            out=yq_t[:, a0:b0], in0=in0,
                            scalar=dwq_sb[s][:, c, t:t + 1],
                            in1=yq_t[:, a0:b0],
                            op0=AOP.mult, op1=AOP.add)
                    yq.append(yq_t)

                    # kv depthwise (stride 2), strided from padded x.  The
                    # pad cols make the column edges automatic; only row
                    # ranges need restricting.  (GPSIMD can't do per-partition
                    # scalar ops - TensorScalarPtr is invalid on Pool.)
                    ykv_t = ykpool.tile([128, HWK], BF16, tag="ykv", name="ykv")
                    k3 = ykv_t[:].rearrange("p (r q) -> p r q", r=16)
                    x3 = xe_t[:].rearrange("p (r c) -> p r c", r=32)
                    nc.vector.tensor_scalar_mul(k3[:, :, :], x3[:, 0:32:2, 1:32:2],
                                                dwkv_sb[s][:, c, 4:5])
                    for t, (ky, kx) in enumerate((ky, kx) for ky in range(3)
                                                 for kx in range(3)):
                        if t == 4:
                            continue
                        r0 = 1 if ky == 0 else 0
                        rlo = 2 * r0 + ky - 1
                        rhi = rlo + 2 * (16 - r0)
                        nc.vector.scalar_tensor_tensor(
                            out=k3[:, r0:16, :],
                            in0=x3[:, rlo:min(rhi, 32):2, kx:kx + 31:2],
                            scalar=dwkv_sb[s][:, c, t:t + 1],
                            in1=k3[:, r0:16, :],
                            op0=AOP.mult, op1=AOP.add)
                    ykv.append(ykv_t)

                # Q pointwise: qT[m] [128, 1024]; rhs reads skip the pad cols
                yv = [yq[k][:].rearrange("p (r c) -> p r c", r=32) for k in range(2)]
                for m in range(4):
                    ps = psB.tile([128, HW], F32, tag="big", name="big")
                    for qh in range(2):
                        for k in range(2):
                            nc.tensor.matmul(
                                out=ps[:, qh * 512:(qh + 1) * 512],
                                lhsT=wq_sb[s][k][:, m * 128:(m + 1) * 128],
                                rhs=yv[k][:, 16 * qh:16 * (qh + 1), 1:33],
                                start=(k == 0), stop=(k == 1))
                    qT_t = qTpool.tile([128, HW], BF16, tag="qT", name="qT")
                    nc.scalar.activation(out=qT_t[:], in_=ps[:], func=AF.Identity,
                                         bias=bq_sb[s][:, m:m + 1], scale=1.0)
                    qT[(s, m)] = qT_t
                # K pointwise: kT[m] [128, 256]
                for m in range(4):
                    ps = psA.tile([128, HWK], F32, tag="mm", name="mm")
                    for k in range(2):
                        nc.tensor.matmul(
                            out=ps[:],
                            lhsT=wkv_sb[s][k][:, m * 128:(m + 1) * 128],
                            rhs=ykv[k][:],
                            start=(k == 0), stop=(k == 1))
                    kT_t = kTpool.tile([128, HWK], BF16, tag="kT", name="kT")
                    nc.scalar.activation(out=kT_t[:], in_=ps[:], func=AF.Identity,
                                         bias=bk_sb[s][:, m:m + 1], scale=1.0)
                    kT[(s, m)] = kT_t
                # V pointwise (x-stationary): v[p] [128 pos, 512 cout]
                for p in range(2):
                    ps = psA.tile([128, 512], F32, tag="mm", name="mm")
                    for k in range(2):
                        nc.tensor.matmul(
                            out=ps[:],
                            lhsT=ykv[k][:, p * 128:(p + 1) * 128],
                            rhs=wkv_sb[s][k][:, INNER:2 * INNER],
                            start=(k == 0), stop=(k == 1))
                    v_t = vpool.tile([128, 512], BF16, tag="v", name="v")
                    nc.vector.tensor_copy(v_t[:], ps[:])
                    v[(s, p)] = v_t

            # ---- attention blocks ------------------------------------------
            for a in range(2):          # output stream a: q from a, k/v from 1-a
                b = 1 - a
                pT = {}
                # dotsT + exp, per head pair
                for hp in range(4):
                    for kc in range(2):
                        dp = [psB.tile([128, HW], F32, tag="big", name="big") for _ in range(2)]
                        for j in range(2):
                            for qh in range(2):
                                nc.tensor.matmul(
                                    out=dp[j][:, qh * 512:(qh + 1) * 512],
                                    lhsT=kT[(b, hp)][64 * j:64 * (j + 1),
                                                     kc * 128:(kc + 1) * 128],
                                    rhs=qT[(a, hp)][64 * j:64 * (j + 1),
                                                    qh * 512:(qh + 1) * 512],
                                    start=True, stop=True)
                        for j in range(2):
                            pT_t = pTpool.tile([128, HW], BF16, tag="pT", name="pT")
                            nc.scalar.activation(out=pT_t[:], in_=dp[j][:],
                                                 func=AF.Exp, scale=SCALE)
                            pT[(2 * hp + j, kc)] = pT_t
                # per pair: denominators (ones-matmuls; even head -> psum rows
                # 0-63, odd head -> rows 64-127), reciprocal off PSUM, then oT
                # with the same direct partition-offset writes; normalization
                # fused into the eviction.  Interleaved per-pair so the
                # reciprocal overlaps the oT matmuls.
                oT = {}
                for hp in range(4):
                    d_t = psD.tile([128, HW], F32, tag="d", name="d")
                    for qh in range(2):
                        for j in range(2):
                            for kc in range(2):
                                nc.tensor.matmul(
                                    out=d_t[64 * j:64 * (j + 1), qh * 512:(qh + 1) * 512],
                                    lhsT=ones_sb[:],
                                    rhs=pT[(2 * hp + j, kc)][:, qh * 512:(qh + 1) * 512],
                                    start=(kc == 0), stop=(kc == 1))
                    dr_t = Rpool.tile([128, HW], F32, tag="R", name="R")
                    nc.vector.reciprocal_approx_fast(out=dr_t[:], in_=d_t[:])
                    po = psB.tile([128, HW], F32, tag="big", name="big")
                    for qh in range(2):
                        for j in range(2):
                            for kc in range(2):
                                nc.tensor.matmul(
                                    out=po[64 * j:64 * (j + 1), qh * 512:(qh + 1) * 512],
                                    lhsT=v[(b, kc)][:, 128 * hp + 64 * j:
                                                    128 * hp + 64 * (j + 1)],
                                    rhs=pT[(2 * hp + j, kc)][:, qh * 512:(qh + 1) * 512],
                                    start=(kc == 0), stop=(kc == 1))
                    oT_t = oTpool.tile([128, HW], BF16, tag="oT", name="oT")
                    nc.vector.tensor_mul(oT_t[:], po[:], dr_t[:])
                    oT[hp] = oT_t
                # output 1x1 conv + bias
                for cc in range(2):
                    out_t = outpool.tile([128, HW], F32, tag="out", name="out")
                    for qh in range(2):
                        ps = psA.tile([128, 512], F32, tag="mm", name="mm")
                        for hp in range(4):
                            nc.tensor.matmul(
                                out=ps[:],
                                lhsT=wo_sb[a][hp][:, cc * 128:(cc + 1) * 128],
                                rhs=oT[hp][:, qh * 512:(qh + 1) * 512],
                                start=(hp == 0), stop=(hp == 3))
                        nc.scalar.activation(
                            out=out_t[:, qh * 512:(qh + 1) * 512], in_=ps[:],
                            func=AF.Identity, bias=bo_sb[a][:, cc:cc + 1], scale=1.0)
                    nc.sync.dma_start(
                        out=out_d.ap()[a, img, cc * 128:(cc + 1) * 128, :],
                        in_=out_t[:])
    nc.compile()
    return nc


_MODULE = None


def _get_module():
    global _MODULE
    if _MODULE is None:
        _MODULE = _build_module()
    return _MODULE


# ---------------------------------------------------------------------------
# host side: BN folding + sharding + launch
# ---------------------------------------------------------------------------

def _fold(inputs, p):
    dw = np.asarray(inputs[p + '_dw'], np.float32)[:, 0]        # [256,3,3]
    g = np.asarray(inputs[p + '_g'], np.float32)
    b_ = np.asarray(inputs[p + '_b'], np.float32)
    rm = np.asarray(inputs[p + '_rm'], np.float32)
    rv = np.asarray(inputs[p + '_rv'], np.float32)
    pw = np.asarray(inputs[p + '_pw'], np.float32)[:, :, 0, 0]  # [cout, 256]
    inv = g / np.sqrt(rv + EPS)
    dw_eff = (dw * inv[:, None, None]).reshape(DIM, 9)
    bias = pw @ (b_ - rm * inv)
    return dw_eff, pw.T.copy(), bias                             # WT [256, cout]


def host_arrays(inputs):
    """Folded per-core-constant DRAM tensors (same on every core)."""
    h = {'ones_in': np.ones((128, 64), NPBF)}
    bv = {}
    for s, qp, kvp in ((0, 'q1', 'kv1'), (1, 'q2', 'kv2')):
        dwq, WqT, bq = _fold(inputs, qp)
        dwkv, WkvT, bkv = _fold(inputs, kvp)
        h[f'dwq{s}'] = dwq.reshape(2, 128, 9)
        h[f'dwkv{s}'] = dwkv.reshape(2, 128, 9)
        h[f'wq{s}'] = np.ascontiguousarray(WqT.astype(NPBF))
        h[f'wkv{s}'] = np.ascontiguousarray(WkvT.astype(NPBF))
        h[f'bq{s}'] = bq.reshape(4, 128)
        h[f'bk{s}'] = bkv[:INNER].reshape(4, 128)
        bv[s] = bkv[INNER:]
    for s, op in ((0, 'out1'), (1, 'out2')):
        Wout = np.asarray(inputs[op + '_w'], np.float32)[:, :, 0, 0]  # [256, 512]
        bo = np.asarray(inputs[op + '_b'], np.float32) + Wout @ bv[1 - s]
        h[f'wo{s}'] = np.ascontiguousarray(Wout.T.astype(NPBF))
        h[f'bo{s}'] = bo.reshape(2, 128)
    out = {}
    for k, a in h.items():
        if a.dtype == NPBF:
            out[k] = np.ascontiguousarray(a)
        else:
            out[k] = np.ascontiguousarray(a, dtype=np.float32)
    return out


def _pad_x(x):
    """[B, 256, 32, 32] fp32 -> [B, 256, 1088] bf16 with zero pad cols."""
    xp = np.zeros((B, DIM, 32, 34), NPBF)
    xp[:, :, :, 1:33] = x.astype(NPBF)
    return xp.reshape(B, DIM, 32 * 34)


def make_in_maps(inputs):
    h = host_arrays(inputs)
    x1 = _pad_x(np.asarray(inputs['x1'], np.float32).reshape(B, DIM, 32, 32))
    x2 = _pad_x(np.asarray(inputs['x2'], np.float32).reshape(B, DIM, 32, 32))
    maps = []
    for c in range(N_CORES):
        m = dict(h)
        m['xs0'] = np.ascontiguousarray(x1[c * IMGS:(c + 1) * IMGS])
        m['xs1'] = np.ascontiguousarray(x2[c * IMGS:(c + 1) * IMGS])
        maps.append(m)
    return maps


def gather_out(core_outs):
    """core_outs: list of [2, IMGS, 256, 1024] -> [2, B, 256, 32, 32]."""
    full = np.concatenate([np.asarray(o) for o in core_outs], axis=1)
    return np.ascontiguousarray(full.reshape(2, B, DIM, 32, 32))


def kernel(**inputs):
    nc = _get_module()
    in_maps = make_in_maps(inputs)
    res = run_bass_kernel_spmd(nc, in_maps, list(range(N_CORES)))
    return gather_out([r['out'] for r in res.results])


if __name__ == '__main__':
    nc = _build_module()
    print("module built OK")


# revision 9
# speedup vs baseline: 1.4415x; 1.4415x over previous
"""ConvCrossAttention Trainium2 kernel (Bass/Tile), SPMD over 8 NeuronCores.

Sharding: pure data-parallel over batch (B=16 -> 2 images per core). Each core
runs the full two-stream cross-attention block for its 2 images; no collectives.

v2: full-fp16 dataflow (fp32r matmuls stream at ~2 cycles/col and trip the PE
power throttle; bf16 streams 1 col/cycle).  All matmul operands are bf16, PSUM
accumulation stays fp32.  Engine placement per the measured v1 trace:

  - depthwise 3x3 q-conv on DVE: x is stored column-padded ([32,34] rows, zero
    pad cols) plus an odd-offset copy, so every tap is a single contiguous 1-D
    scalar_tensor_tensor with 4B-aligned bf16 operands -> DVE 2x packed mode,
    no per-row turnaround.
  - depthwise stride-2 kv-conv on GPSIMD (strided reads defeat DVE packing;
    GPSIMD is otherwise idle).
  - pointwise convs on PE as before (Q: weight-stationary into [cout,pos],
    K: weight-stationary, V: x-stationary into [pos,cout]).
  - attention per head pair: dotsT on PE, exp on ACT (psum->sbuf bf16),
    denominator via ones-matmuls writing even head to psum rows 0-63 and odd
    head to rows 64-127 directly (no zero-padded weights), reciprocal on DVE,
    oT on PE with the same direct partition-offset writes (no zeroed v copy),
    normalization fused into the PSUM->SBUF eviction (DVE mult -> bf16).
  - psum->sbuf bias evictions (qT, kT, out) on ACT, which sits closer to PSUM.
  - V conv bias folded into the output conv bias on the host; BN folded into
    depthwise taps + pointwise bias.
"""

import numpy as np
import ml_dtypes
from contextlib import ExitStack

import concourse.bass as bass
import concourse.bacc as bacc
import concourse.tile as tile
import concourse.mybir as mybir
from concourse.bass_utils import run_bass_kernel_spmd

F32 = mybir.dt.float32
BF16 = mybir.dt.float16
AOP = mybir.AluOpType
AF = mybir.ActivationFunctionType

N_CORES = 8
B = 16
IMGS = B // N_CORES          # images per core
DIM = 256                    # conv in channels
HEADS, DH, INNER = 8, 64, 512
HW = 1024                    # 32*32 q positions
HWP = 1088                   # 32*34 column-padded positions
HWK = 256                    # 16*16 kv positions
EPS = 1e-5
SCALE = DH ** -0.5

NPBF = np.float16


# ---------------------------------------------------------------------------
# device kernel
# ---------------------------------------------------------------------------

def _build_module():
    nc = bacc.Bacc("TRN2", target_bir_lowering=False, debug=False)

    def inp(name, shape, dt=F32):
        return nc.dram_tensor(name, shape, dt, kind="ExternalInput")

    x_d = [inp(f"xs{s}", [IMGS, DIM, HWP], BF16) for s in range(2)]
    dwq_d = [inp(f"dwq{s}", [2, 128, 9]) for s in range(2)]
    dwkv_d = [inp(f"dwkv{s}", [2, 128, 9]) for s in range(2)]
    wq_d = [inp(f"wq{s}", [DIM, INNER], BF16) for s in range(2)]        # [cin, cout]
    wkv_d = [inp(f"wkv{s}", [DIM, 2 * INNER], BF16) for s in range(2)]
    bq_d = [inp(f"bq{s}", [4, 128]) for s in range(2)]                  # cout chunk-major
    bk_d = [inp(f"bk{s}", [4, 128]) for s in range(2)]
    wo_d = [inp(f"wo{s}", [INNER, DIM], BF16) for s in range(2)]        # [hd, c]
    bo_d = [inp(f"bo{s}", [2, 128]) for s in range(2)]
    ones_d = inp("ones_in", [128, 64], BF16)
    dq_d = [inp(f"dq{s}", [2, 128, 9 * 128], BF16) for s in range(2)]
    out_d = nc.dram_tensor("out", [2, IMGS, DIM, HW], F32, kind="ExternalOutput")

    with tile.TileContext(nc) as tc, ExitStack() as ctx:
        const = ctx.enter_context(tc.tile_pool(name="const", bufs=1))
        xepool = ctx.enter_context(tc.tile_pool(name="xepool", bufs=2))
        yqpool = ctx.enter_context(tc.tile_pool(name="yqpool", bufs=2))
        ykpool = ctx.enter_context(tc.tile_pool(name="ykpool", bufs=2))
        qTpool = ctx.enter_context(tc.tile_pool(name="qTpool", bufs=8))
        kTpool = ctx.enter_context(tc.tile_pool(name="kTpool", bufs=8))
        vpool = ctx.enter_context(tc.tile_pool(name="vpool", bufs=4))
        pTpool = ctx.enter_context(tc.tile_pool(name="pTpool", bufs=16))
        Rpool = ctx.enter_context(tc.tile_pool(name="Rpool", bufs=3))
        oTpool = ctx.enter_context(tc.tile_pool(name="oTpool", bufs=4))
        outpool = ctx.enter_context(tc.tile_pool(name="outpool", bufs=2))
        psA = ctx.enter_context(tc.tile_pool(name="psA", bufs=2, space="PSUM"))
        psD = ctx.enter_context(tc.tile_pool(name="psD", bufs=1, space="PSUM"))
        psB = ctx.enter_context(tc.tile_pool(name="psB", bufs=2, space="PSUM"))

        # ---- constants -----------------------------------------------------
        wq_sb, wkv_sb, wo_sb, dwq_sb, dwkv_sb, bq_sb, bk_sb, bo_sb = (
            [], [], [], [], [], [], [], [])
        for s in range(2):
            wq_sb.append([const.tile([128, INNER], BF16, tag=f"wq{s}{k}", name=f"wq{s}{k}")
                          for k in range(2)])
            wkv_sb.append([const.tile([128, 2 * INNER], BF16, tag=f"wkv{s}{k}", name=f"wkv{s}{k}")
                           for k in range(2)])
            wo_sb.append([const.tile([128, DIM], BF16, tag=f"wo{s}{m}", name=f"wo{s}{m}")
                          for m in range(4)])
            for k in range(2):
                nc.sync.dma_start(out=wq_sb[s][k][:],
                                  in_=wq_d[s].ap()[k * 128:(k + 1) * 128, :])
                nc.sync.dma_start(out=wkv_sb[s][k][:],
                                  in_=wkv_d[s].ap()[k * 128:(k + 1) * 128, :])
            for m in range(4):
                nc.sync.dma_start(out=wo_sb[s][m][:],
                                  in_=wo_d[s].ap()[m * 128:(m + 1) * 128, :])
            dwq_sb.append([const.tile([128, 9 * 128], BF16, tag=f"dq{s}{c}", name=f"dq{s}{c}")
                           for c in range(2)])
            for c in range(2):
                nc.sync.dma_start(out=dwq_sb[s][c][:], in_=dq_d[s].ap()[c])
            dwkv_sb.append(const.tile([128, 2, 9], F32, tag=f"dwkv{s}", name=f"dwkv{s}"))
            nc.sync.dma_start(out=dwkv_sb[s][:],
                              in_=dwkv_d[s].ap().rearrange("c p t -> p c t"))
            bq_sb.append(const.tile([128, 4], F32, tag=f"bq{s}", name=f"bq{s}"))
            bk_sb.append(const.tile([128, 4], F32, tag=f"bk{s}", name=f"bk{s}"))
            bo_sb.append(const.tile([128, 2], F32, tag=f"bo{s}", name=f"bo{s}"))
            nc.sync.dma_start(out=bq_sb[s][:], in_=bq_d[s].ap().rearrange("m p -> p m"))
            nc.sync.dma_start(out=bk_sb[s][:], in_=bk_d[s].ap().rearrange("m p -> p m"))
            nc.sync.dma_start(out=bo_sb[s][:], in_=bo_d[s].ap().rearrange("m p -> p m"))
        ones_sb = const.tile([128, 64], BF16, tag="ones", name="ones")
        nc.sync.dma_start(out=ones_sb[:], in_=ones_d.ap())

        for img in range(IMGS):
            qT, kT, v = {}, {}, {}
            # ---- projections for both streams ------------------------------
            for s in range(2):
                yq, ykv = [], []
                # tap order: identity first (start=True must cover all rows),
                # a full-row tap (ky=1) last for the stop
                torder = [4, 0, 1, 2, 6, 7, 8, 5, 3]
                for c in range(2):
                    xe_t = xepool.tile([128, HWP], BF16, tag="xe", name="xe")
                    nc.sync.dma_start(out=xe_t[:],
                                      in_=x_d[s].ap()[img, c * 128:(c + 1) * 128, :])
                    x3 = xe_t[:].rearrange("p (r c) -> p r c", r=32)
                    # q depthwise on PE: 9 diagonal matmuls accumulating in psum
                    y_ps = psB.tile([128, HW], F32, tag="big", name="big")
                    y3 = y_ps.rearrange("p (r c) -> p r c", r=32)
                    for t in torder:
                        ky, kx = t // 3, t % 3
                        r0 = 1 if ky == 0 else 0
                        r1 = 31 if ky == 2 else 32
                        for qh in range(2):
                            lo, hi = max(r0, 16 * qh), min(r1, 16 * qh + 16)
                            nc.tensor.matmul(
                                out=y3[:, lo:hi, :],
                                lhsT=dwq_sb[s][c][:, t * 128:(t + 1) * 128],
                                rhs=x3[:, lo + ky - 1:hi + ky - 1, kx:kx + 32],
                                start=(t == 4), stop=(t == 3))
                    yq_t = yqpool.tile([128, HW], BF16, tag="yq", name="yq")
                    nc.scalar.activation(out=yq_t[:], in_=y_ps[:], func=AF.Identity,
                                         scale=1.0)
                    yq.append(yq_t)

                    # kv depthwise (stride 2) on DVE, strided from padded x.
                    # Pad cols make the column edges automatic.
                    ykv_t = ykpool.tile([128, HWK], BF16, tag="ykv", name="ykv")
                    k3 = ykv_t[:].rearrange("p (r q) -> p r q", r=16)
                    nc.vector.tensor_scalar_mul(k3[:, :, :], x3[:, 0:32:2, 1:32:2],
                                                dwkv_sb[s][:, c, 4:5])
                    for t, (ky, kx) in enumerate((ky, kx) for ky in range(3)
                                                 for kx in range(3)):
                        if t == 4:
                            continue
                        r0 = 1 if ky == 0 else 0
                        rlo = 2 * r0 + ky - 1
                        rhi = rlo + 2 * (16 - r0)
                        nc.vector.scalar_tensor_tensor(
                            out=k3[:, r0:16, :],
                            in0=x3[:, rlo:min(rhi, 32):2, kx:kx + 31:2],
                            scalar=dwkv_sb[s][:, c, t:t + 1],
                            in1=k3[:, r0:16, :],
                            op0=AOP.mult, op1=AOP.add)
                    ykv.append(ykv_t)

                # Q pointwise: qT[m] [128, 1024]
                for m in range(4):
                    ps = psB.tile([128, HW], F32, tag="big", name="big")
                    for qh in range(2):
                        for k in range(2):
                            nc.tensor.matmul(
                                out=ps[:, qh * 512:(qh + 1) * 512],
                                lhsT=wq_sb[s][k][:, m * 128:(m + 1) * 128],
                                rhs=yq[k][:, qh * 512:(qh + 1) * 512],
                                start=(k == 0), stop=(k == 1))
                    qT_t = qTpool.tile([128, HW], BF16, tag="qT", name="qT")
                    if m < 2:
                        nc.scalar.activation(out=qT_t[:], in_=ps[:], func=AF.Identity,
                                             bias=bq_sb[s][:, m:m + 1], scale=1.0)
                    else:
                        nc.vector.tensor_scalar_add(qT_t[:], ps[:],
                                                    bq_sb[s][:, m:m + 1])
                    qT[(s, m)] = qT_t
                # K pointwise: kT[m] [128, 256]
                for m in range(4):
                    ps = psA.tile([128, HWK], F32, tag="mm", name="mm")
                    for k in range(2):
                        nc.tensor.matmul(
                            out=ps[:],
                            lhsT=wkv_sb[s][k][:, m * 128:(m + 1) * 128],
                            rhs=ykv[k][:],
                            start=(k == 0), stop=(k == 1))
                    kT_t = kTpool.tile([128, HWK], BF16, tag="kT", name="kT")
                    nc.scalar.activation(out=kT_t[:], in_=ps[:], func=AF.Identity,
                                         bias=bk_sb[s][:, m:m + 1], scale=1.0)
                    kT[(s, m)] = kT_t
                # V pointwise (x-stationary): v[p] [128 pos, 512 cout]
                for p in range(2):
                    ps = psA.tile([128, 512], F32, tag="mm", name="mm")
                    for k in range(2):
                        nc.tensor.matmul(
                            out=ps[:],
                            lhsT=ykv[k][:, p * 128:(p + 1) * 128],
                            rhs=wkv_sb[s][k][:, INNER:2 * INNER],
                            start=(k == 0), stop=(k == 1))
                    v_t = vpool.tile([128, 512], BF16, tag="v", name="v")
                    nc.vector.tensor_copy(v_t[:], ps[:])
                    v[(s, p)] = v_t

            # ---- attention blocks ------------------------------------------
            for a in range(2):          # output stream a: q from a, k/v from 1-a
                b = 1 - a
                pT = {}
                # dotsT + exp, per head pair
                for hp in range(4):
                    for kc in range(2):
                        dp = [psB.tile([128, HW], F32, tag="big", name="big") for _ in range(2)]
                        for j in range(2):
                            for qh in range(2):
                                nc.tensor.matmul(
                                    out=dp[j][:, qh * 512:(qh + 1) * 512],
                                    lhsT=kT[(b, hp)][64 * j:64 * (j + 1),
                                                     kc * 128:(kc + 1) * 128],
                                    rhs=qT[(a, hp)][64 * j:64 * (j + 1),
                                                    qh * 512:(qh + 1) * 512],
                                    start=True, stop=True)
                        for j in range(2):
                            pT_t = pTpool.tile([128, HW], BF16, tag="pT", name="pT")
                            nc.scalar.activation(out=pT_t[:], in_=dp[j][:],
                                                 func=AF.Exp, scale=SCALE)
                            pT[(2 * hp + j, kc)] = pT_t
                # per pair: denominators (ones-matmuls; even head -> psum rows
                # 0-63, odd head -> rows 64-127), reciprocal off PSUM, then oT
                # with the same direct partition-offset writes; normalization
                # fused into the eviction.  Interleaved per-pair so the
                # reciprocal overlaps the oT matmuls.
                oT = {}
                for hp in range(4):
                    d_t = psD.tile([128, HW], F32, tag="d", name="d")
                    for qh in range(2):
                        for j in range(2):
                            for kc in range(2):
                                nc.tensor.matmul(
                                    out=d_t[64 * j:64 * (j + 1), qh * 512:(qh + 1) * 512],
                                    lhsT=ones_sb[:],
                                    rhs=pT[(2 * hp + j, kc)][:, qh * 512:(qh + 1) * 512],
                                    start=(kc == 0), stop=(kc == 1))
                    dr_t = Rpool.tile([128, HW], F32, tag="R", name="R")
                    nc.vector.reciprocal_approx_fast(out=dr_t[:], in_=d_t[:])
                    po = psB.tile([128, HW], F32, tag="big", name="big")
                    for qh in range(2):
                        for j in range(2):
                            for kc in range(2):
                                nc.tensor.matmul(
                                    out=po[64 * j:64 * (j + 1), qh * 512:(qh + 1) * 512],
                                    lhsT=v[(b, kc)][:, 128 * hp + 64 * j:
                                                    128 * hp + 64 * (j + 1)],
                                    rhs=pT[(2 * hp + j, kc)][:, qh * 512:(qh + 1) * 512],
                                    start=(kc == 0), stop=(kc == 1))
                    oT_t = oTpool.tile([128, HW], BF16, tag="oT", name="oT")
                    nc.vector.tensor_mul(oT_t[:], po[:], dr_t[:])
                    oT[hp] = oT_t
                # output 1x1 conv + bias
                for cc in range(2):
                    out_t = outpool.tile([128, HW], F32, tag="out", name="out")
                    for qh in range(2):
                        ps = psA.tile([128, 512], F32, tag="mm", name="mm")
                        for hp in range(4):
                            nc.tensor.matmul(
                                out=ps[:],
                                lhsT=wo_sb[a][hp][:, cc * 128:(cc + 1) * 128],
                                rhs=oT[hp][:, qh * 512:(qh + 1) * 512],
                                start=(hp == 0), stop=(hp == 3))
                        nc.scalar.activation(
                            out=out_t[:, qh * 512:(qh + 1) * 512], in_=ps[:],
                            func=AF.Identity, bias=bo_sb[a][:, cc:cc + 1], scale=1.0)
                    nc.sync.dma_start(
                        out=out_d.ap()[a, img, cc * 128:(cc + 1) * 128, :],
                        in_=out_t[:])
    nc.compile()
    return nc


_MODULE = None


def _get_module():
    global _MODULE
    if _MODULE is None:
        _MODULE = _build_module()
    return _MODULE


# ---------------------------------------------------------------------------
# host side: BN folding + sharding + launch
# ---------------------------------------------------------------------------

def _fold(inputs, p):
    dw = np.asarray(inputs[p + '_dw'], np.float32)[:, 0]        # [256,3,3]
    g = np.asarray(inputs[p + '_g'], np.float32)
    b_ = np.asarray(inputs[p + '_b'], np.float32)
    rm = np.asarray(inputs[p + '_rm'], np.float32)
    rv = np.asarray(inputs[p + '_rv'], np.float32)
    pw = np.asarray(inputs[p + '_pw'], np.float32)[:, :, 0, 0]  # [cout, 256]
    inv = g / np.sqrt(rv + EPS)
    dw_eff = (dw * inv[:, None, None]).reshape(DIM, 9)
    bias = pw @ (b_ - rm * inv)
    return dw_eff, pw.T.copy(), bias                             # WT [256, cout]


def host_arrays(inputs):
    """Folded per-core-constant DRAM tensors (same on every core)."""
    h = {'ones_in': np.ones((128, 64), NPBF)}
    bv = {}
    for s, qp, kvp in ((0, 'q1', 'kv1'), (1, 'q2', 'kv2')):
        dwq, WqT, bq = _fold(inputs, qp)
        dwkv, WkvT, bkv = _fold(inputs, kvp)
        h[f'dwq{s}'] = dwq.reshape(2, 128, 9)
        h[f'dwkv{s}'] = dwkv.reshape(2, 128, 9)
        dq = np.zeros((2, 128, 9, 128), NPBF)
        for c in range(2):
            r = np.arange(128)
            dq[c, r, :, r] = dwq[c * 128:(c + 1) * 128, :].astype(NPBF)
        h[f'dq{s}'] = dq.reshape(2, 128, 9 * 128)
        h[f'wq{s}'] = np.ascontiguousarray(WqT.astype(NPBF))
        h[f'wkv{s}'] = np.ascontiguousarray(WkvT.astype(NPBF))
        h[f'bq{s}'] = bq.reshape(4, 128)
        h[f'bk{s}'] = bkv[:INNER].reshape(4, 128)
        bv[s] = bkv[INNER:]
    for s, op in ((0, 'out1'), (1, 'out2')):
        Wout = np.asarray(inputs[op + '_w'], np.float32)[:, :, 0, 0]  # [256, 512]
        bo = np.asarray(inputs[op + '_b'], np.float32) + Wout @ bv[1 - s]
        h[f'wo{s}'] = np.ascontiguousarray(Wout.T.astype(NPBF))
        h[f'bo{s}'] = bo.reshape(2, 128)
    out = {}
    for k, a in h.items():
        if a.dtype == NPBF:
            out[k] = np.ascontiguousarray(a)
        else:
            out[k] = np.ascontiguousarray(a, dtype=np.float32)
    return out


def _pad_x(x):
    """[B, 256, 32, 32] fp32 -> [B, 256, 1088] bf16 with zero pad cols."""
    xp = np.zeros((B, DIM, 32, 34), NPBF)
    xp[:, :, :, 1:33] = x.astype(NPBF)
    return xp.reshape(B, DIM, 32 * 34)


def make_in_maps(inputs):
    h = host_arrays(inputs)
    x1 = _pad_x(np.asarray(inputs['x1'], np.float32).reshape(B, DIM, 32, 32))
    x2 = _pad_x(np.asarray(inputs['x2'], np.float32).reshape(B, DIM, 32, 32))
    maps = []
    for c in range(N_CORES):
        m = dict(h)
        m['xs0'] = np.ascontiguousarray(x1[c * IMGS:(c + 1) * IMGS])
        m['xs1'] = np.ascontiguousarray(x2[c * IMGS:(c + 1) * IMGS])
        maps.append(m)
    return maps


def gather_out(core_outs):
    """core_outs: list of [2, IMGS, 256, 1024] -> [2, B, 256, 32, 32]."""
    full = np.concatenate([np.asarray(o) for o in core_outs], axis=1)
    return np.ascontiguousarray(full.reshape(2, B, DIM, 32, 32))


def kernel(**inputs):
    nc = _get_module()
    in_maps = make_in_maps(inputs)
    res = run_bass_kernel_spmd(nc, in_maps, list(range(N_CORES)))
    return gather_out([r['out'] for r in res.results])


if __name__ == '__main__':
    nc = _build_module()
    print("module built OK")


# revision 11
# speedup vs baseline: 1.5128x; 1.0495x over previous
"""ConvCrossAttention Trainium2 kernel (Bass/Tile), SPMD over 8 NeuronCores.

Sharding: pure data-parallel over batch (B=16 -> 2 images per core). Each core
runs the full two-stream cross-attention block for its 2 images; no collectives.

v2: full-fp16 dataflow (fp32r matmuls stream at ~2 cycles/col and trip the PE
power throttle; bf16 streams 1 col/cycle).  All matmul operands are bf16, PSUM
accumulation stays fp32.  Engine placement per the measured v1 trace:

  - depthwise 3x3 q-conv on DVE: x is stored column-padded ([32,34] rows, zero
    pad cols) plus an odd-offset copy, so every tap is a single contiguous 1-D
    scalar_tensor_tensor with 4B-aligned bf16 operands -> DVE 2x packed mode,
    no per-row turnaround.
  - depthwise stride-2 kv-conv on GPSIMD (strided reads defeat DVE packing;
    GPSIMD is otherwise idle).
  - pointwise convs on PE as before (Q: weight-stationary into [cout,pos],
    K: weight-stationary, V: x-stationary into [pos,cout]).
  - attention per head pair: dotsT on PE, exp on ACT (psum->sbuf bf16),
    denominator via ones-matmuls writing even head to psum rows 0-63 and odd
    head to rows 64-127 directly (no zero-padded weights), reciprocal on DVE,
    oT on PE with the same direct partition-offset writes (no zeroed v copy),
    normalization fused into the PSUM->SBUF eviction (DVE mult -> bf16).
  - psum->sbuf bias evictions (qT, kT, out) on ACT, which sits closer to PSUM.
  - V conv bias folded into the output conv bias on the host; BN folded into
    depthwise taps + pointwise bias.
"""

import numpy as np
import ml_dtypes
from contextlib import ExitStack

import concourse.bass as bass
import concourse.bacc as bacc
import concourse.tile as tile
import concourse.mybir as mybir
from concourse.bass_utils import run_bass_kernel_spmd

F32 = mybir.dt.float32
BF16 = mybir.dt.float16
AOP = mybir.AluOpType
AF = mybir.ActivationFunctionType

N_CORES = 8
B = 16
IMGS = B // N_CORES          # images per core
DIM = 256                    # conv in channels
HEADS, DH, INNER = 8, 64, 512
HW = 1024                    # 32*32 q positions
HWP = 1088                   # 32*34 column-padded positions
HWK = 256                    # 16*16 kv positions
EPS = 1e-5
SCALE = DH ** -0.5

NPBF = np.float16


# ---------------------------------------------------------------------------
# device kernel
# ---------------------------------------------------------------------------

def _build_module():
    nc = bacc.Bacc("TRN2", target_bir_lowering=False, debug=False)

    def inp(name, shape, dt=F32):
        return nc.dram_tensor(name, shape, dt, kind="ExternalInput")

    x_d = [inp(f"xs{s}", [IMGS, DIM, HWP], BF16) for s in range(2)]
    dwq_d = [inp(f"dwq{s}", [2, 128, 9]) for s in range(2)]
    dwkv_d = [inp(f"dwkv{s}", [2, 128, 9]) for s in range(2)]
    wq_d = [inp(f"wq{s}", [DIM, INNER], BF16) for s in range(2)]        # [cin, cout]
    wkv_d = [inp(f"wkv{s}", [DIM, 2 * INNER], BF16) for s in range(2)]
    bq_d = [inp(f"bq{s}", [4, 128]) for s in range(2)]                  # cout chunk-major
    bk_d = [inp(f"bk{s}", [4, 128]) for s in range(2)]
    wo_d = [inp(f"wo{s}", [INNER, DIM], BF16) for s in range(2)]        # [hd, c]
    bo_d = [inp(f"bo{s}", [2, 128]) for s in range(2)]
    ones_d = inp("ones_in", [128, 64], BF16)
    dq_d = [inp(f"dq{s}", [2, 128, 9 * 128], BF16) for s in range(2)]
    out_d = nc.dram_tensor("out", [2, IMGS, DIM, HW], F32, kind="ExternalOutput")

    with tile.TileContext(nc) as tc, ExitStack() as ctx:
        const = ctx.enter_context(tc.tile_pool(name="const", bufs=1))
        xepool = ctx.enter_context(tc.tile_pool(name="xepool", bufs=2))
        yqpool = ctx.enter_context(tc.tile_pool(name="yqpool", bufs=2))
        ykpool = ctx.enter_context(tc.tile_pool(name="ykpool", bufs=2))
        qTpool = ctx.enter_context(tc.tile_pool(name="qTpool", bufs=8))
        kTpool = ctx.enter_context(tc.tile_pool(name="kTpool", bufs=8))
        vpool = ctx.enter_context(tc.tile_pool(name="vpool", bufs=4))
        pTpool = ctx.enter_context(tc.tile_pool(name="pTpool", bufs=32))
        Rpool = ctx.enter_context(tc.tile_pool(name="Rpool", bufs=3))
        oTpool = ctx.enter_context(tc.tile_pool(name="oTpool", bufs=8))
        outpool = ctx.enter_context(tc.tile_pool(name="outpool", bufs=2))
        psA = ctx.enter_context(tc.tile_pool(name="psA", bufs=2, space="PSUM"))
        psD = ctx.enter_context(tc.tile_pool(name="psD", bufs=1, space="PSUM"))
        psB = ctx.enter_context(tc.tile_pool(name="psB", bufs=2, space="PSUM"))

        # ---- constants (tap-critical dq/dwkv/bias loads first; the bulky
        # proj weights last - they are not needed until ~20us in) -----------
        wq_sb, wkv_sb, wo_sb, dwq_sb, dwkv_sb, bq_sb, bk_sb, bo_sb = (
            [], [], [], [], [], [], [], [])
        for s in range(2):
            dwq_sb.append([const.tile([128, 9 * 128], BF16, tag=f"dq{s}{c}", name=f"dq{s}{c}")
                           for c in range(2)])
            for c in range(2):
                nc.sync.dma_start(out=dwq_sb[s][c][:], in_=dq_d[s].ap()[c])
            dwkv_sb.append(const.tile([128, 2, 9], F32, tag=f"dwkv{s}", name=f"dwkv{s}"))
            nc.sync.dma_start(out=dwkv_sb[s][:],
                              in_=dwkv_d[s].ap().rearrange("c p t -> p c t"))
            bq_sb.append(const.tile([128, 4], F32, tag=f"bq{s}", name=f"bq{s}"))
            bk_sb.append(const.tile([128, 4], F32, tag=f"bk{s}", name=f"bk{s}"))
            bo_sb.append(const.tile([128, 2], F32, tag=f"bo{s}", name=f"bo{s}"))
            nc.sync.dma_start(out=bq_sb[s][:], in_=bq_d[s].ap().rearrange("m p -> p m"))
            nc.sync.dma_start(out=bk_sb[s][:], in_=bk_d[s].ap().rearrange("m p -> p m"))
            nc.sync.dma_start(out=bo_sb[s][:], in_=bo_d[s].ap().rearrange("m p -> p m"))
        ones_sb = const.tile([128, 64], BF16, tag="ones", name="ones")
        nc.sync.dma_start(out=ones_sb[:], in_=ones_d.ap())
        for s in range(2):
            wq_sb.append([const.tile([128, INNER], BF16, tag=f"wq{s}{k}", name=f"wq{s}{k}")
                          for k in range(2)])
            wkv_sb.append([const.tile([128, 2 * INNER], BF16, tag=f"wkv{s}{k}", name=f"wkv{s}{k}")
                           for k in range(2)])
            wo_sb.append([const.tile([128, DIM], BF16, tag=f"wo{s}{m}", name=f"wo{s}{m}")
                          for m in range(4)])
            for k in range(2):
                nc.sync.dma_start(out=wq_sb[s][k][:],
                                  in_=wq_d[s].ap()[k * 128:(k + 1) * 128, :])
                nc.sync.dma_start(out=wkv_sb[s][k][:],
                                  in_=wkv_d[s].ap()[k * 128:(k + 1) * 128, :])
            for m in range(4):
                nc.sync.dma_start(out=wo_sb[s][m][:],
                                  in_=wo_d[s].ap()[m * 128:(m + 1) * 128, :])

        for img in range(IMGS):
            qT, kT, v = {}, {}, {}
            # ---- projections for both streams ------------------------------
            for s in range(2):
                yq, ykv = [], []
                # tap order: identity first (start=True must cover all rows),
                # a full-row tap (ky=1) last for the stop
                torder = [4, 0, 1, 2, 6, 7, 8, 5, 3]
                for c in range(2):
                    xe_t = xepool.tile([128, HWP], BF16, tag="xe", name="xe")
                    nc.gpsimd.dma_start(out=xe_t[:],
                                        in_=x_d[s].ap()[img, c * 128:(c + 1) * 128, :])
                    x3 = xe_t[:].rearrange("p (r c) -> p r c", r=32)
                    # q depthwise on PE: 9 diagonal matmuls accumulating in psum
                    y_ps = psB.tile([128, HW], F32, tag="big", name="big")
                    y3 = y_ps.rearrange("p (r c) -> p r c", r=32)
                    for t in torder:
                        ky, kx = t // 3, t % 3
                        r0 = 1 if ky == 0 else 0
                        r1 = 31 if ky == 2 else 32
                        for qh in range(2):
                            lo, hi = max(r0, 16 * qh), min(r1, 16 * qh + 16)
                            nc.tensor.matmul(
                                out=y3[:, lo:hi, :],
                                lhsT=dwq_sb[s][c][:, t * 128:(t + 1) * 128],
                                rhs=x3[:, lo + ky - 1:hi + ky - 1, kx:kx + 32],
                                start=(t == 4), stop=(t == 3))
                    yq_t = yqpool.tile([128, HW], BF16, tag="yq", name="yq")
                    nc.scalar.activation(out=yq_t[:], in_=y_ps[:], func=AF.Identity,
                                         scale=1.0)
                    yq.append(yq_t)

                    # kv depthwise (stride 2) on DVE, strided from padded x.
                    # Pad cols make the column edges automatic.
                    ykv_t = ykpool.tile([128, HWK], BF16, tag="ykv", name="ykv")
                    k3 = ykv_t[:].rearrange("p (r q) -> p r q", r=16)
                    nc.vector.tensor_scalar_mul(k3[:, :, :], x3[:, 0:32:2, 1:32:2],
                                                dwkv_sb[s][:, c, 4:5])
                    for t, (ky, kx) in enumerate((ky, kx) for ky in range(3)
                                                 for kx in range(3)):
                        if t == 4:
                            continue
                        r0 = 1 if ky == 0 else 0
                        rlo = 2 * r0 + ky - 1
                        rhi = rlo + 2 * (16 - r0)
                        nc.vector.scalar_tensor_tensor(
                            out=k3[:, r0:16, :],
                            in0=x3[:, rlo:min(rhi, 32):2, kx:kx + 31:2],
                            scalar=dwkv_sb[s][:, c, t:t + 1],
                            in1=k3[:, r0:16, :],
                            op0=AOP.mult, op1=AOP.add)
                    ykv.append(ykv_t)

                # Q pointwise: qT[m] [128, 1024]
                for m in range(4):
                    ps = psB.tile([128, HW], F32, tag="big", name="big")
                    for qh in range(2):
                        for k in range(2):
                            nc.tensor.matmul(
                                out=ps[:, qh * 512:(qh + 1) * 512],
                                lhsT=wq_sb[s][k][:, m * 128:(m + 1) * 128],
                                rhs=yq[k][:, qh * 512:(qh + 1) * 512],
                                start=(k == 0), stop=(k == 1))
                    qT_t = qTpool.tile([128, HW], BF16, tag="qT", name="qT")
                    if m < 2:
                        nc.scalar.activation(out=qT_t[:], in_=ps[:], func=AF.Identity,
                                             bias=bq_sb[s][:, m:m + 1], scale=1.0)
                    else:
                        nc.vector.tensor_scalar_add(qT_t[:], ps[:],
                                                    bq_sb[s][:, m:m + 1])
                    qT[(s, m)] = qT_t
                # K pointwise: kT[m] [128, 256]
                for m in range(4):
                    ps = psA.tile([128, HWK], F32, tag="mm", name="mm")
                    for k in range(2):
                        nc.tensor.matmul(
                            out=ps[:],
                            lhsT=wkv_sb[s][k][:, m * 128:(m + 1) * 128],
                            rhs=ykv[k][:],
                            start=(k == 0), stop=(k == 1))
                    kT_t = kTpool.tile([128, HWK], BF16, tag="kT", name="kT")
                    nc.scalar.activation(out=kT_t[:], in_=ps[:], func=AF.Identity,
                                         bias=bk_sb[s][:, m:m + 1], scale=1.0)
                    kT[(s, m)] = kT_t
                # V pointwise (x-stationary): v[p] [128 pos, 512 cout]
                for p in range(2):
                    ps = psA.tile([128, 512], F32, tag="mm", name="mm")
                    for k in range(2):
                        nc.tensor.matmul(
                            out=ps[:],
                            lhsT=ykv[k][:, p * 128:(p + 1) * 128],
                            rhs=wkv_sb[s][k][:, INNER:2 * INNER],
                            start=(k == 0), stop=(k == 1))
                    v_t = vpool.tile([128, 512], BF16, tag="v", name="v")
                    nc.vector.tensor_copy(v_t[:], ps[:])
                    v[(s, p)] = v_t

            # ---- attention blocks ------------------------------------------
            # Both streams' dots+exp first (exps of a=0 complete behind the
            # dots of a=1), then denom/oT per pair, then the output convs.
            pT, dr, oT = {}, {}, {}
            for a in range(2):          # output stream a: q from a, k/v from 1-a
                b = 1 - a
                for hp in range(4):
                    for kc in range(2):
                        dp = [psB.tile([128, HW], F32, tag="big", name="big") for _ in range(2)]
                        for j in range(2):
                            for qh in range(2):
                                nc.tensor.matmul(
                                    out=dp[j][:, qh * 512:(qh + 1) * 512],
                                    lhsT=kT[(b, hp)][64 * j:64 * (j + 1),
                                                     kc * 128:(kc + 1) * 128],
                                    rhs=qT[(a, hp)][64 * j:64 * (j + 1),
                                                    qh * 512:(qh + 1) * 512],
                                    start=True, stop=True)
                        for j in range(2):
                            pT_t = pTpool.tile([128, HW], BF16, tag="pT", name="pT")
                            nc.scalar.activation(out=pT_t[:], in_=dp[j][:],
                                                 func=AF.Exp, scale=SCALE)
                            pT[(a, 2 * hp + j, kc)] = pT_t
            for a in range(2):
                b = 1 - a
                # per pair: denominators (ones-matmuls; even head -> psum rows
                # 0-63, odd head -> rows 64-127), reciprocal off PSUM, then oT
                # with the same direct partition-offset writes; normalization
                # fused into the eviction.
                for hp in range(4):
                    d_t = psD.tile([128, HW], F32, tag="d", name="d")
                    for qh in range(2):
                        for j in range(2):
                            for kc in range(2):
                                nc.tensor.matmul(
                                    out=d_t[64 * j:64 * (j + 1), qh * 512:(qh + 1) * 512],
                                    lhsT=ones_sb[:],
                                    rhs=pT[(a, 2 * hp + j, kc)][:, qh * 512:(qh + 1) * 512],
                                    start=(kc == 0), stop=(kc == 1))
                    dr_t = Rpool.tile([128, HW], F32, tag="R", name="R")
                    nc.vector.reciprocal_approx_fast(out=dr_t[:], in_=d_t[:])
                    po = psB.tile([128, HW], F32, tag="big", name="big")
                    for qh in range(2):
                        for j in range(2):
                            for kc in range(2):
                                nc.tensor.matmul(
                                    out=po[64 * j:64 * (j + 1), qh * 512:(qh + 1) * 512],
                                    lhsT=v[(b, kc)][:, 128 * hp + 64 * j:
                                                    128 * hp + 64 * (j + 1)],
                                    rhs=pT[(a, 2 * hp + j, kc)][:, qh * 512:(qh + 1) * 512],
                                    start=(kc == 0), stop=(kc == 1))
                    oT_t = oTpool.tile([128, HW], BF16, tag="oT", name="oT")
                    nc.vector.tensor_mul(oT_t[:], po[:], dr_t[:])
                    oT[(a, hp)] = oT_t
            for a in range(2):
                # output 1x1 conv + bias
                for cc in range(2):
                    out_t = outpool.tile([128, HW], F32, tag="out", name="out")
                    for qh in range(2):
                        ps = psA.tile([128, 512], F32, tag="mm", name="mm")
                        for hp in range(4):
                            nc.tensor.matmul(
                                out=ps[:],
                                lhsT=wo_sb[a][hp][:, cc * 128:(cc + 1) * 128],
                                rhs=oT[(a, hp)][:, qh * 512:(qh + 1) * 512],
                                start=(hp == 0), stop=(hp == 3))
                        nc.scalar.activation(
                            out=out_t[:, qh * 512:(qh + 1) * 512], in_=ps[:],
                            func=AF.Identity, bias=bo_sb[a][:, cc:cc + 1], scale=1.0)
                    nc.sync.dma_start(
                        out=out_d.ap()[a, img, cc * 128:(cc + 1) * 128, :],
                        in_=out_t[:])
    nc.compile()
    return nc


_MODULE = None


def _get_module():
    global _MODULE
    if _MODULE is None:
        _MODULE = _build_module()
    return _MODULE


# ---------------------------------------------------------------------------
# host side: BN folding + sharding + launch
# ---------------------------------------------------------------------------

def _fold(inputs, p):
    dw = np.asarray(inputs[p + '_dw'], np.float32)[:, 0]        # [256,3,3]
    g = np.asarray(inputs[p + '_g'], np.float32)
    b_ = np.asarray(inputs[p + '_b'], np.float32)
    rm = np.asarray(inputs[p + '_rm'], np.float32)
    rv = np.asarray(inputs[p + '_rv'], np.float32)
    pw = np.asarray(inputs[p + '_pw'], np.float32)[:, :, 0, 0]  # [cout, 256]
    inv = g / np.sqrt(rv + EPS)
    dw_eff = (dw * inv[:, None, None]).reshape(DIM, 9)
    bias = pw @ (b_ - rm * inv)
    return dw_eff, pw.T.copy(), bias                             # WT [256, cout]


def host_arrays(inputs):
    """Folded per-core-constant DRAM tensors (same on every core)."""
    h = {'ones_in': np.ones((128, 64), NPBF)}
    bv = {}
    for s, qp, kvp in ((0, 'q1', 'kv1'), (1, 'q2', 'kv2')):
        dwq, WqT, bq = _fold(inputs, qp)
        dwkv, WkvT, bkv = _fold(inputs, kvp)
        h[f'dwq{s}'] = dwq.reshape(2, 128, 9)
        h[f'dwkv{s}'] = dwkv.reshape(2, 128, 9)
        dq = np.zeros((2, 128, 9, 128), NPBF)
        for c in range(2):
            r = np.arange(128)
            dq[c, r, :, r] = dwq[c * 128:(c + 1) * 128, :].astype(NPBF)
        h[f'dq{s}'] = dq.reshape(2, 128, 9 * 128)
        h[f'wq{s}'] = np.ascontiguousarray(WqT.astype(NPBF))
        h[f'wkv{s}'] = np.ascontiguousarray(WkvT.astype(NPBF))
        h[f'bq{s}'] = bq.reshape(4, 128)
        h[f'bk{s}'] = bkv[:INNER].reshape(4, 128)
        bv[s] = bkv[INNER:]
    for s, op in ((0, 'out1'), (1, 'out2')):
        Wout = np.asarray(inputs[op + '_w'], np.float32)[:, :, 0, 0]  # [256, 512]
        bo = np.asarray(inputs[op + '_b'], np.float32) + Wout @ bv[1 - s]
        h[f'wo{s}'] = np.ascontiguousarray(Wout.T.astype(NPBF))
        h[f'bo{s}'] = bo.reshape(2, 128)
    out = {}
    for k, a in h.items():
        if a.dtype == NPBF:
            out[k] = np.ascontiguousarray(a)
        else:
            out[k] = np.ascontiguousarray(a, dtype=np.float32)
    return out


def _pad_x(x):
    """[B, 256, 32, 32] fp32 -> [B, 256, 1088] bf16 with zero pad cols."""
    xp = np.zeros((B, DIM, 32, 34), NPBF)
    xp[:, :, :, 1:33] = x.astype(NPBF)
    return xp.reshape(B, DIM, 32 * 34)


def make_in_maps(inputs):
    h = host_arrays(inputs)
    x1 = _pad_x(np.asarray(inputs['x1'], np.float32).reshape(B, DIM, 32, 32))
    x2 = _pad_x(np.asarray(inputs['x2'], np.float32).reshape(B, DIM, 32, 32))
    maps = []
    for c in range(N_CORES):
        m = dict(h)
        m['xs0'] = np.ascontiguousarray(x1[c * IMGS:(c + 1) * IMGS])
        m['xs1'] = np.ascontiguousarray(x2[c * IMGS:(c + 1) * IMGS])
        maps.append(m)
    return maps


def gather_out(core_outs):
    """core_outs: list of [2, IMGS, 256, 1024] -> [2, B, 256, 32, 32]."""
    full = np.concatenate([np.asarray(o) for o in core_outs], axis=1)
    return np.ascontiguousarray(full.reshape(2, B, DIM, 32, 32))


def kernel(**inputs):
    nc = _get_module()
    in_maps = make_in_maps(inputs)
    res = run_bass_kernel_spmd(nc, in_maps, list(range(N_CORES)))
    return gather_out([r['out'] for r in res.results])


if __name__ == '__main__':
    nc = _build_module()
    print("module built OK")
